# revision 12
# baseline (speedup 1.0000x reference)
"""Trainium2 Bass kernel for nn_DecomposeModel (gated 2-layer MLP decompose).

Strategy:
  - Host: sort rows by group. Only group==0 rows need the left GateNN,
    only group==1 rows need the right GateNN, group==2 rows output zero.
    Deal g0/g1 rows round-robin across the 8 cores (data parallel), pad
    each segment to a fixed per-core cap so all cores run one SPMD program.
  - Device: activations kept transposed [feature, row] so every matmul
    uses the weights in native [in, out] layout as the stationary operand
    (out = W_tile.T @ xT_tile). bf16 matmuls, f32 PSUM accumulation,
    tanh/sigmoid on ScalarE with fused bias, gating product on VectorE.
    Input x-stream DMAs ride the Sync HWDGE ring, weights ride the
    Scalar HWDGE ring, outputs ride GpSimd SWDGE — three independent
    issue paths so the weight preload doesn't stall the first blocks.
  - Host: scatter device outputs back to full [B, H] f32 (g2 rows stay 0).
"""

import sys

try:
    import concourse  # noqa: F401
except ImportError:
    sys.path.insert(0, "/opt/trn_rl_repo")

import numpy as np
import ml_dtypes

import concourse.tile as tile
from concourse import bacc, mybir
from concourse.bass_utils import run_bass_kernel_spmd

B = 32768
H = 512
NCORES = 8
BC = B // NCORES  # per-core shard of the mask output
DEFAULT_CAP = 1408  # per-core per-branch row capacity (B/3/8 = 1365.3 avg)

BF16 = mybir.dt.bfloat16
F32 = mybir.dt.float32
I32 = mybir.dt.int32

# biases stacked [8, 512] in this order
BIAS_ORDER = ["bl1h", "bl1g", "bl2h", "bl2g", "br1h", "br1g", "br2h", "br2g"]

_PROGRAM_CACHE = {}
LAST_RESULT = None  # BassKernelResults of the most recent kernel() call


def _blocks(cap):
    out = []
    rem = cap
    while rem > 0:
        b = 512 if rem >= 576 else rem
        out.append(b)
        rem -= b
    return out


def build_program(cap0, cap1):
    """Emit + compile the SPMD program for per-branch caps (cap0, cap1)."""
    nc = bacc.Bacc("TRN2", target_bir_lowering=False, debug=False,
                   num_devices=NCORES)

    ncols = cap0 + cap1
    x1t_d = nc.dram_tensor("x1t", [3 * H, ncols], BF16, kind="ExternalInput")
    lt_d = nc.dram_tensor("lt", [H, cap1], BF16, kind="ExternalInput")
    grp_d = nc.dram_tensor("grp", [BC], I32, kind="ExternalInput")
    bias_d = nc.dram_tensor("bias8", [8, H], F32, kind="ExternalInput")

    w_shapes = [("wl1h", 3 * H), ("wl1g", 3 * H),
                ("wl2h", H), ("wl2g", H),
                ("wr1h", 3 * H), ("wr1g", 3 * H),
                ("wr2h", 2 * H), ("wr2g", 2 * H)]
    w_d = {}
    for name, kdim in w_shapes:
        w_d[name] = nc.dram_tensor(name, [kdim, H], BF16, kind="ExternalInput")

    outt_d = nc.dram_tensor("outt", [H, ncols], F32, kind="ExternalOutput")
    fin_d = nc.dram_tensor("fin", [BC], I32, kind="ExternalOutput")

    with tile.TileContext(nc) as tc:
        with (
            tc.tile_pool(name="wsb", bufs=1) as wpool,
            tc.tile_pool(name="bsb", bufs=1) as bpool,
            tc.tile_pool(name="xsb", bufs=3) as xpool,
            tc.tile_pool(name="hsb", bufs=2) as hpool,
            tc.tile_pool(name="act", bufs=3) as apool,
            tc.tile_pool(name="osb", bufs=6) as opool,
            tc.tile_pool(name="msc", bufs=1) as mpool,
            tc.tile_pool(name="ps", bufs=8, space="PSUM") as pspool,
        ):
            # --- persistent weights / biases -------------------------------
            # Weights ride the Scalar HWDGE ring, the x-stream rides the
            # Sync ring, outputs ride GpSimd SWDGE. The first matmuls only
            # gate on bias8 + first half of wl1h + first half of x-block-0;
            # explicit deps below keep later DMAs from stealing HBM
            # bandwidth during that window.
            b_sb = bpool.tile([128, 8, 4], F32, tag="bias8")
            nc.scalar.dma_start(
                out=b_sb[:], in_=bias_d.rearrange("b (j p) -> p b j", p=128)
            )
            bias_ap = {n: b_sb[:, i, :] for i, n in enumerate(BIAS_ORDER)}

            # wl1h is split in two kt-halves so the very first matmuls can
            # start after ~1.5 MB instead of ~3 MB of DMA.
            w_sb = {}      # name -> list of (tile, kt_start, nkt)
            w_dma = {}     # name -> last dma instruction
            for name, kdim in w_shapes:
                nk = kdim // 128
                if name == "wl1h":
                    ta = wpool.tile([128, nk // 2, H], BF16, tag="w_wl1h_a")
                    tb = wpool.tile([128, nk - nk // 2, H], BF16,
                                    tag="w_wl1h_b")
                    w_sb[name] = [(ta, 0, nk // 2), (tb, nk // 2, nk)]
                else:
                    t = wpool.tile([128, nk, H], BF16, tag=f"w_{name}")
                    w_sb[name] = [(t, 0, nk)]

            def _load_w(name):
                src = w_d[name].rearrange("(kt p) f -> p kt f", p=128)
                for t, k0, k1 in w_sb[name]:
                    w_dma[name] = nc.scalar.dma_start(
                        out=t[:], in_=src[:, k0:k1, :])
                return w_dma[name]

            def _wsl(name, kt, ft):
                for t, k0, k1 in w_sb[name]:
                    if k0 <= kt < k1:
                        return t[:, kt - k0, ft * 128:(ft + 1) * 128]
                raise AssertionError

            _d_wl1h = _load_w("wl1h")

            # --- finished mask (GpSimd SWDGE) ------------------------------
            gt = mpool.tile([128, BC // 128], I32, tag="grp")
            nc.gpsimd.dma_start(
                out=gt[:], in_=grp_d.rearrange("(p j) -> p j", p=128)
            )
            ft_ = mpool.tile([128, BC // 128], I32, tag="fin")
            nc.vector.tensor_scalar(ft_[:], gt[:], 2, None,
                                    op0=mybir.AluOpType.is_equal)
            nc.gpsimd.dma_start(
                out=fin_d.rearrange("(p j) -> p j", p=128), in_=ft_[:]
            )

            x1t_r = x1t_d.rearrange("(kt p) n -> p kt n", p=128)
            lt_r = lt_d.rearrange("(kt p) n -> p kt n", p=128)
            outt_r = outt_d.rearrange("(ft p) n -> p ft n", p=128)

            def branch(col0, cap, w1h, w1g, b1h, b1g, w2h, w2g, b2h, b2g,
                       with_lt, deferred_w=(), split_first=False):
                deferred_w = list(deferred_w)
                c0 = 0
                first = True
                for rblk in _blocks(cap):
                    cs = slice(col0 + c0, col0 + c0 + rblk)
                    if split_first and first:
                        xa = xpool.tile([128, 6, rblk], BF16, tag="xa")
                        nc.sync.dma_start(out=xa[:], in_=x1t_r[:, 0:6, cs])
                        xb = xpool.tile([128, 6, rblk], BF16, tag="xb")
                        dxl = nc.sync.dma_start(out=xb[:],
                                                in_=x1t_r[:, 6:12, cs])
                        xparts = [(xa, 0, 6), (xb, 6, 12)]
                        _st["x0_dma"] = dxl
                    else:
                        x_sb = xpool.tile([128, 12, rblk], BF16, tag="x")
                        dxl = nc.sync.dma_start(out=x_sb[:],
                                                in_=x1t_r[:, :, cs])
                        if _st.get("hold_x"):
                            tile.add_dep_helper(
                                dxl.ins, _st.pop("hold_x").ins, sync=True,
                                reason="defer x prefetch behind wl1h load")
                        xparts = [(x_sb, 0, 12)]
                    first = False

                    def _xsl(kt):
                        for t, k0, k1 in xparts:
                            if k0 <= kt < k1:
                                return t[:, kt - k0, :]
                        raise AssertionError

                    while deferred_w:
                        wname = deferred_w.pop(0)
                        dw = _load_w(wname)
                        if _st.get("x0_dma"):
                            tile.add_dep_helper(
                                dw.ins, _st.pop("x0_dma").ins, sync=True,
                                reason="defer weight loads behind x block 0")

                    h_sb = hpool.tile([128, 4, rblk], BF16, tag="h")
                    for ft in range(4):
                        ph = pspool.tile([128, rblk], F32, tag="ps")
                        for kt in range(12):
                            nc.tensor.matmul(
                                ph[:], _wsl(w1h, kt, ft), _xsl(kt),
                                start=(kt == 0), stop=(kt == 11),
                            )
                        pg = pspool.tile([128, rblk], F32, tag="ps")
                        for kt in range(12):
                            nc.tensor.matmul(
                                pg[:], _wsl(w1g, kt, ft), _xsl(kt),
                                start=(kt == 0), stop=(kt == 11),
                            )
                        th = apool.tile([128, rblk], BF16, tag="th")
                        sg = apool.tile([128, rblk], BF16, tag="sg")
                        nc.scalar.activation(
                            th[:], ph[:], mybir.ActivationFunctionType.Tanh,
                            bias=b1h[:, ft:ft + 1])
                        nc.scalar.activation(
                            sg[:], pg[:], mybir.ActivationFunctionType.Sigmoid,
                            bias=b1g[:, ft:ft + 1])
                        nc.vector.tensor_mul(h_sb[:, ft, :], th[:], sg[:])

                    if with_lt:
                        lt_sb = xpool.tile([128, 4, rblk], BF16, tag="ltx")
                        nc.sync.dma_start(
                            out=lt_sb[:],
                            in_=lt_r[:, :, c0: c0 + rblk],
                        )
                    nk2 = 8 if with_lt else 4
                    for ft in range(4):
                        ph = pspool.tile([128, rblk], F32, tag="ps")
                        for kt in range(nk2):
                            rhs = (h_sb[:, kt, :] if kt < 4
                                   else lt_sb[:, kt - 4, :])
                            nc.tensor.matmul(
                                ph[:], _wsl(w2h, kt, ft), rhs,
                                start=(kt == 0), stop=(kt == nk2 - 1),
                            )
                        pg = pspool.tile([128, rblk], F32, tag="ps")
                        for kt in range(nk2):
                            rhs = (h_sb[:, kt, :] if kt < 4
                                   else lt_sb[:, kt - 4, :])
                            nc.tensor.matmul(
                                pg[:], _wsl(w2g, kt, ft), rhs,
                                start=(kt == 0), stop=(kt == nk2 - 1),
                            )
                        th = apool.tile([128, rblk], F32, tag="th2")
                        sg = apool.tile([128, rblk], F32, tag="sg2")
                        nc.scalar.activation(
                            th[:], ph[:], mybir.ActivationFunctionType.Tanh,
                            bias=b2h[:, ft:ft + 1])
                        nc.scalar.activation(
                            sg[:], pg[:], mybir.ActivationFunctionType.Sigmoid,
                            bias=b2g[:, ft:ft + 1])
                        o_sb = opool.tile([128, rblk], F32, tag="o")
                        nc.vector.tensor_mul(o_sb[:], th[:], sg[:])
                        nc.gpsimd.dma_start(
                            out=outt_r[:, ft, cs],
                            in_=o_sb[:],
                        )
                    c0 += rblk

            _st = {"hold_x": _d_wl1h}
            branch(0, cap0,
                   "wl1h", "wl1g", bias_ap["bl1h"], bias_ap["bl1g"],
                   "wl2h", "wl2g", bias_ap["bl2h"], bias_ap["bl2g"],
                   with_lt=False, deferred_w=["wl1g", "wl2h", "wl2g"],
                   split_first=True)
            branch(cap0, cap1,
                   "wr1h", "wr1g", bias_ap["br1h"], bias_ap["br1g"],
                   "wr2h", "wr2g", bias_ap["br2h"], bias_ap["br2g"],
                   with_lt=True,
                   deferred_w=["wr1h", "wr1g", "wr2h", "wr2g"])

    nc.compile()
    return nc


def _get_program(cap0, cap1):
    key = (cap0, cap1)
    if key not in _PROGRAM_CACHE:
        _PROGRAM_CACHE[key] = build_program(cap0, cap1)
    return _PROGRAM_CACHE[key]


def _roundup(x, m):
    return ((x + m - 1) // m) * m


def kernel(node_hidden, node_context, label_embedding, left_embedding, group,
           Wl1h, bl1h, Wl1g, bl1g, Wl2h, bl2h, Wl2g, bl2g,
           Wr1h, br1h, Wr1g, br1g, Wr2h, br2h, Wr2g, br2g,
           trace=False, trace_kwargs=None):
    global LAST_RESULT
    group = np.asarray(group)
    idx0 = np.flatnonzero(group == 0)
    idx1 = np.flatnonzero(group == 1)
    per0 = [idx0[c::NCORES] for c in range(NCORES)]
    per1 = [idx1[c::NCORES] for c in range(NCORES)]
    need0 = max(len(p) for p in per0)
    need1 = max(len(p) for p in per1)
    cap0 = DEFAULT_CAP if need0 <= DEFAULT_CAP else _roundup(need0, 64)
    cap1 = DEFAULT_CAP if need1 <= DEFAULT_CAP else _roundup(need1, 64)

    nc = _get_program(cap0, cap1)

    bf = ml_dtypes.bfloat16
    xcat = np.concatenate(
        [np.asarray(node_hidden), np.asarray(node_context),
         np.asarray(label_embedding)], axis=1)  # [B, 3H] f32
    lemb = np.asarray(left_embedding)

    shared = {
        "wl1h": np.ascontiguousarray(Wl1h).astype(bf),
        "wl1g": np.ascontiguousarray(Wl1g).astype(bf),
        "wr1h": np.ascontiguousarray(Wr1h).astype(bf),
        "wr1g": np.ascontiguousarray(Wr1g).astype(bf),
        "wl2h": np.ascontiguousarray(Wl2h).astype(bf),
        "wl2g": np.ascontiguousarray(Wl2g).astype(bf),
        "wr2h": np.ascontiguousarray(Wr2h).astype(bf),
        "wr2g": np.ascontiguousarray(Wr2g).astype(bf),
        "bias8": np.ascontiguousarray(np.stack(
            [bl1h, bl1g, bl2h, bl2g, br1h, br1g, br2h, br2g]),
            dtype=np.float32),
    }

    in_maps = []
    for c in range(NCORES):
        rows0 = np.zeros(cap0, dtype=np.int64)
        rows0[:len(per0[c])] = per0[c]
        rows1 = np.zeros(cap1, dtype=np.int64)
        rows1[:len(per1[c])] = per1[c]
        rows = np.concatenate([rows0, rows1])
        x1t = np.ascontiguousarray(xcat[rows].T).astype(bf)  # [3H, ncols]
        lt = np.ascontiguousarray(lemb[rows1].T).astype(bf)  # [H, cap1]
        m = dict(shared)
        m["x1t"] = x1t
        m["lt"] = lt
        m["grp"] = np.ascontiguousarray(group[c * BC:(c + 1) * BC],
                                        dtype=np.int32)
        in_maps.append(m)

    res = run_bass_kernel_spmd(nc, in_maps, list(range(NCORES)),
                               trace=trace, **(trace_kwargs or {}))
    LAST_RESULT = res

    children = np.zeros((B, H), dtype=np.float32)
    finished = np.empty(B, dtype=np.int32)
    for c in range(NCORES):
        outt = res.results[c]["outt"]  # [H, ncols] f32
        outr = outt.T  # [ncols, H]
        if len(per0[c]):
            children[per0[c]] = outr[:len(per0[c])]
        if len(per1[c]):
            children[per1[c]] = outr[cap0:cap0 + len(per1[c])]
        finished[c * BC:(c + 1) * BC] = res.results[c]["fin"]
    return children, finished


# revision 13
# speedup vs baseline: 1.0088x; 1.0088x over previous
"""Trainium2 Bass kernel for nn_DecomposeModel (gated 2-layer MLP decompose).

Strategy:
  - Host: sort rows by group. Only group==0 rows need the left GateNN,
    only group==1 rows need the right GateNN, group==2 rows output zero.
    Deal g0/g1 rows round-robin across the 8 cores (data parallel), pad
    each segment to a fixed per-core cap so all cores run one SPMD program.
  - Device: activations kept transposed [feature, row] so every matmul
    uses the weights in native [in, out] layout as the stationary operand
    (out = W_tile.T @ xT_tile). bf16 matmuls, f32 PSUM accumulation,
    tanh/sigmoid on ScalarE with fused bias, gating product on VectorE.
    Input x-stream DMAs ride the Sync HWDGE ring, weights ride the
    Scalar HWDGE ring, outputs ride GpSimd SWDGE — three independent
    issue paths so the weight preload doesn't stall the first blocks.
  - Host: scatter device outputs back to full [B, H] f32 (g2 rows stay 0).
"""

import sys

try:
    import concourse  # noqa: F401
except ImportError:
    sys.path.insert(0, "/opt/trn_rl_repo")

import numpy as np
import ml_dtypes

import concourse.tile as tile
from concourse import bacc, mybir
from concourse.bass_utils import run_bass_kernel_spmd

B = 32768
H = 512
NCORES = 8
BC = B // NCORES  # per-core shard of the mask output
DEFAULT_CAP = 1408  # per-core per-branch row capacity (B/3/8 = 1365.3 avg)

BF16 = mybir.dt.bfloat16
F32 = mybir.dt.float32
I32 = mybir.dt.int32

# biases stacked [8, 512] in this order
BIAS_ORDER = ["bl1h", "bl1g", "bl2h", "bl2g", "br1h", "br1g", "br2h", "br2g"]

_PROGRAM_CACHE = {}
LAST_RESULT = None  # BassKernelResults of the most recent kernel() call


def _blocks(cap):
    out = []
    rem = cap
    while rem > 0:
        b = 512 if rem >= 576 else rem
        out.append(b)
        rem -= b
    return out


def build_program(cap0, cap1):
    """Emit + compile the SPMD program for per-branch caps (cap0, cap1)."""
    nc = bacc.Bacc("TRN2", target_bir_lowering=False, debug=False,
                   num_devices=NCORES)

    ncols = cap0 + cap1
    x1t_d = nc.dram_tensor("x1t", [3 * H, ncols], BF16, kind="ExternalInput")
    lt_d = nc.dram_tensor("lt", [H, cap1], BF16, kind="ExternalInput")
    grp_d = nc.dram_tensor("grp", [BC], I32, kind="ExternalInput")
    bias_d = nc.dram_tensor("bias8", [8, H], F32, kind="ExternalInput")

    w_shapes = [("wl1h", 3 * H), ("wl1g", 3 * H),
                ("wl2h", H), ("wl2g", H),
                ("wr1h", 3 * H), ("wr1g", 3 * H),
                ("wr2h", 2 * H), ("wr2g", 2 * H)]
    w_d = {}
    for name, kdim in w_shapes:
        w_d[name] = nc.dram_tensor(name, [kdim, H], BF16, kind="ExternalInput")

    outt_d = nc.dram_tensor("outt", [H, ncols], F32, kind="ExternalOutput")
    fin_d = nc.dram_tensor("fin", [BC], I32, kind="ExternalOutput")

    with tile.TileContext(nc) as tc:
        with (
            tc.tile_pool(name="wsb", bufs=1) as wpool,
            tc.tile_pool(name="bsb", bufs=1) as bpool,
            tc.tile_pool(name="xsb", bufs=3) as xpool,
            tc.tile_pool(name="hsb", bufs=2) as hpool,
            tc.tile_pool(name="act", bufs=3) as apool,
            tc.tile_pool(name="osb", bufs=6) as opool,
            tc.tile_pool(name="msc", bufs=1) as mpool,
            tc.tile_pool(name="ps", bufs=8, space="PSUM") as pspool,
        ):
            # --- persistent weights / biases -------------------------------
            # Weights ride the Scalar HWDGE ring, the x-stream rides the
            # Sync ring, outputs ride GpSimd SWDGE. The first matmuls only
            # gate on bias8 + first half of wl1h + first half of x-block-0;
            # explicit deps below keep later DMAs from stealing HBM
            # bandwidth during that window.
            b_sb = bpool.tile([128, 8, 4], F32, tag="bias8")
            nc.scalar.dma_start(
                out=b_sb[:], in_=bias_d.rearrange("b (j p) -> p b j", p=128)
            )
            bias_ap = {n: b_sb[:, i, :] for i, n in enumerate(BIAS_ORDER)}

            # wl1h is split in two kt-halves so the very first matmuls can
            # start after ~1.5 MB instead of ~3 MB of DMA.
            w_sb = {}      # name -> list of (tile, kt_start, nkt)
            w_dma = {}     # name -> last dma instruction
            for name, kdim in w_shapes:
                nk = kdim // 128
                if name == "wl1h":
                    ta = wpool.tile([128, nk // 2, H], BF16, tag="w_wl1h_a")
                    tb = wpool.tile([128, nk - nk // 2, H], BF16,
                                    tag="w_wl1h_b")
                    w_sb[name] = [(ta, 0, nk // 2), (tb, nk // 2, nk)]
                else:
                    t = wpool.tile([128, nk, H], BF16, tag=f"w_{name}")
                    w_sb[name] = [(t, 0, nk)]

            def _load_w(name):
                src = w_d[name].rearrange("(kt p) f -> p kt f", p=128)
                for t, k0, k1 in w_sb[name]:
                    w_dma[name] = nc.scalar.dma_start(
                        out=t[:], in_=src[:, k0:k1, :])
                return w_dma[name]

            def _wsl(name, kt, ft):
                for t, k0, k1 in w_sb[name]:
                    if k0 <= kt < k1:
                        return t[:, kt - k0, ft * 128:(ft + 1) * 128]
                raise AssertionError

            _d_wl1h = _load_w("wl1h")

            # --- finished mask (GpSimd SWDGE) ------------------------------
            gt = mpool.tile([128, BC // 128], I32, tag="grp")
            nc.gpsimd.dma_start(
                out=gt[:], in_=grp_d.rearrange("(p j) -> p j", p=128)
            )
            ft_ = mpool.tile([128, BC // 128], I32, tag="fin")
            nc.vector.tensor_scalar(ft_[:], gt[:], 2, None,
                                    op0=mybir.AluOpType.is_equal)
            nc.gpsimd.dma_start(
                out=fin_d.rearrange("(p j) -> p j", p=128), in_=ft_[:]
            )

            x1t_r = x1t_d.rearrange("(kt p) n -> p kt n", p=128)
            lt_r = lt_d.rearrange("(kt p) n -> p kt n", p=128)
            outt_r = outt_d.rearrange("(ft p) n -> p ft n", p=128)

            def branch(col0, cap, w1h, w1g, b1h, b1g, w2h, w2g, b2h, b2g,
                       with_lt, deferred_w=(), split_first=False):
                deferred_w = list(deferred_w)
                c0 = 0
                first = True
                for rblk in _blocks(cap):
                    cs = slice(col0 + c0, col0 + c0 + rblk)
                    if split_first and first:
                        xa = xpool.tile([128, 6, rblk], BF16, tag="xa")
                        nc.sync.dma_start(out=xa[:], in_=x1t_r[:, 0:6, cs])
                        xb = xpool.tile([128, 6, rblk], BF16, tag="xb")
                        dxl = nc.sync.dma_start(out=xb[:],
                                                in_=x1t_r[:, 6:12, cs])
                        xparts = [(xa, 0, 6), (xb, 6, 12)]
                        _st["x0_dma"] = dxl
                    else:
                        x_sb = xpool.tile([128, 12, rblk], BF16, tag="x")
                        dxl = nc.sync.dma_start(out=x_sb[:],
                                                in_=x1t_r[:, :, cs])
                        if _st.get("hold_x"):
                            tile.add_dep_helper(
                                dxl.ins, _st["hold_x"].ins, sync=True,
                                reason="defer x prefetch behind wl1h load")
                        xparts = [(x_sb, 0, 12)]
                    first = False

                    def _xsl(kt):
                        for t, k0, k1 in xparts:
                            if k0 <= kt < k1:
                                return t[:, kt - k0, :]
                        raise AssertionError

                    while deferred_w:
                        wname = deferred_w.pop(0)
                        dw = _load_w(wname)
                        if _st.get("x0_dma"):
                            tile.add_dep_helper(
                                dw.ins, _st.pop("x0_dma").ins, sync=True,
                                reason="defer weight loads behind x block 0")

                    h_sb = hpool.tile([128, 4, rblk], BF16, tag="h")
                    for ft in range(4):
                        ph = pspool.tile([128, rblk], F32, tag="ps")
                        for kt in range(12):
                            nc.tensor.matmul(
                                ph[:], _wsl(w1h, kt, ft), _xsl(kt),
                                start=(kt == 0), stop=(kt == 11),
                            )
                        pg = pspool.tile([128, rblk], F32, tag="ps")
                        for kt in range(12):
                            nc.tensor.matmul(
                                pg[:], _wsl(w1g, kt, ft), _xsl(kt),
                                start=(kt == 0), stop=(kt == 11),
                            )
                        th = apool.tile([128, rblk], BF16, tag="th")
                        sg = apool.tile([128, rblk], BF16, tag="sg")
                        nc.scalar.activation(
                            th[:], ph[:], mybir.ActivationFunctionType.Tanh,
                            bias=b1h[:, ft:ft + 1])
                        nc.scalar.activation(
                            sg[:], pg[:], mybir.ActivationFunctionType.Sigmoid,
                            bias=b1g[:, ft:ft + 1])
                        nc.vector.tensor_mul(h_sb[:, ft, :], th[:], sg[:])

                    if with_lt:
                        lt_sb = xpool.tile([128, 4, rblk], BF16, tag="ltx")
                        dlt = nc.sync.dma_start(
                            out=lt_sb[:],
                            in_=lt_r[:, :, c0: c0 + rblk],
                        )
                        if _st.get("hold_x"):
                            tile.add_dep_helper(
                                dlt.ins, _st["hold_x"].ins, sync=True,
                                reason="defer lt prefetch behind wl1h load")
                    nk2 = 8 if with_lt else 4
                    for ft in range(4):
                        ph = pspool.tile([128, rblk], F32, tag="ps")
                        for kt in range(nk2):
                            rhs = (h_sb[:, kt, :] if kt < 4
                                   else lt_sb[:, kt - 4, :])
                            nc.tensor.matmul(
                                ph[:], _wsl(w2h, kt, ft), rhs,
                                start=(kt == 0), stop=(kt == nk2 - 1),
                            )
                        pg = pspool.tile([128, rblk], F32, tag="ps")
                        for kt in range(nk2):
                            rhs = (h_sb[:, kt, :] if kt < 4
                                   else lt_sb[:, kt - 4, :])
                            nc.tensor.matmul(
                                pg[:], _wsl(w2g, kt, ft), rhs,
                                start=(kt == 0), stop=(kt == nk2 - 1),
                            )
                        th = apool.tile([128, rblk], F32, tag="th2")
                        sg = apool.tile([128, rblk], F32, tag="sg2")
                        nc.scalar.activation(
                            th[:], ph[:], mybir.ActivationFunctionType.Tanh,
                            bias=b2h[:, ft:ft + 1])
                        nc.scalar.activation(
                            sg[:], pg[:], mybir.ActivationFunctionType.Sigmoid,
                            bias=b2g[:, ft:ft + 1])
                        o_sb = opool.tile([128, rblk], F32, tag="o")
                        nc.vector.tensor_mul(o_sb[:], th[:], sg[:])
                        nc.gpsimd.dma_start(
                            out=outt_r[:, ft, cs],
                            in_=o_sb[:],
                        )
                    c0 += rblk

            _st = {"hold_x": _d_wl1h}
            branch(0, cap0,
                   "wl1h", "wl1g", bias_ap["bl1h"], bias_ap["bl1g"],
                   "wl2h", "wl2g", bias_ap["bl2h"], bias_ap["bl2g"],
                   with_lt=False, deferred_w=["wl1g", "wl2h", "wl2g"],
                   split_first=True)
            branch(cap0, cap1,
                   "wr1h", "wr1g", bias_ap["br1h"], bias_ap["br1g"],
                   "wr2h", "wr2g", bias_ap["br2h"], bias_ap["br2g"],
                   with_lt=True,
                   deferred_w=["wr1h", "wr1g", "wr2h", "wr2g"])

    nc.compile()
    return nc


def _get_program(cap0, cap1):
    key = (cap0, cap1)
    if key not in _PROGRAM_CACHE:
        _PROGRAM_CACHE[key] = build_program(cap0, cap1)
    return _PROGRAM_CACHE[key]


def _roundup(x, m):
    return ((x + m - 1) // m) * m


def kernel(node_hidden, node_context, label_embedding, left_embedding, group,
           Wl1h, bl1h, Wl1g, bl1g, Wl2h, bl2h, Wl2g, bl2g,
           Wr1h, br1h, Wr1g, br1g, Wr2h, br2h, Wr2g, br2g,
           trace=False, trace_kwargs=None):
    global LAST_RESULT
    group = np.asarray(group)
    idx0 = np.flatnonzero(group == 0)
    idx1 = np.flatnonzero(group == 1)
    per0 = [idx0[c::NCORES] for c in range(NCORES)]
    per1 = [idx1[c::NCORES] for c in range(NCORES)]
    need0 = max(len(p) for p in per0)
    need1 = max(len(p) for p in per1)
    cap0 = DEFAULT_CAP if need0 <= DEFAULT_CAP else _roundup(need0, 64)
    cap1 = DEFAULT_CAP if need1 <= DEFAULT_CAP else _roundup(need1, 64)

    nc = _get_program(cap0, cap1)

    bf = ml_dtypes.bfloat16
    xcat = np.concatenate(
        [np.asarray(node_hidden), np.asarray(node_context),
         np.asarray(label_embedding)], axis=1)  # [B, 3H] f32
    lemb = np.asarray(left_embedding)

    shared = {
        "wl1h": np.ascontiguousarray(Wl1h).astype(bf),
        "wl1g": np.ascontiguousarray(Wl1g).astype(bf),
        "wr1h": np.ascontiguousarray(Wr1h).astype(bf),
        "wr1g": np.ascontiguousarray(Wr1g).astype(bf),
        "wl2h": np.ascontiguousarray(Wl2h).astype(bf),
        "wl2g": np.ascontiguousarray(Wl2g).astype(bf),
        "wr2h": np.ascontiguousarray(Wr2h).astype(bf),
        "wr2g": np.ascontiguousarray(Wr2g).astype(bf),
        "bias8": np.ascontiguousarray(np.stack(
            [bl1h, bl1g, bl2h, bl2g, br1h, br1g, br2h, br2g]),
            dtype=np.float32),
    }

    in_maps = []
    for c in range(NCORES):
        rows0 = np.zeros(cap0, dtype=np.int64)
        rows0[:len(per0[c])] = per0[c]
        rows1 = np.zeros(cap1, dtype=np.int64)
        rows1[:len(per1[c])] = per1[c]
        rows = np.concatenate([rows0, rows1])
        x1t = np.ascontiguousarray(xcat[rows].T).astype(bf)  # [3H, ncols]
        lt = np.ascontiguousarray(lemb[rows1].T).astype(bf)  # [H, cap1]
        m = dict(shared)
        m["x1t"] = x1t
        m["lt"] = lt
        m["grp"] = np.ascontiguousarray(group[c * BC:(c + 1) * BC],
                                        dtype=np.int32)
        in_maps.append(m)

    res = run_bass_kernel_spmd(nc, in_maps, list(range(NCORES)),
                               trace=trace, **(trace_kwargs or {}))
    LAST_RESULT = res

    children = np.zeros((B, H), dtype=np.float32)
    finished = np.empty(B, dtype=np.int32)
    for c in range(NCORES):
        outt = res.results[c]["outt"]  # [H, ncols] f32
        outr = outt.T  # [ncols, H]
        if len(per0[c]):
            children[per0[c]] = outr[:len(per0[c])]
        if len(per1[c]):
            children[per1[c]] = outr[cap0:cap0 + len(per1[c])]
        finished[c * BC:(c + 1) * BC] = res.results[c]["fin"]
    return children, finished


# revision 15
# speedup vs baseline: 1.0400x; 1.0310x over previous
"""Trainium2 Bass kernel for nn_DecomposeModel (gated 2-layer MLP decompose).

Strategy:
  - Host: sort rows by group. Only group==0 rows need the left GateNN,
    only group==1 rows need the right GateNN, group==2 rows output zero.
    Deal g0/g1 rows round-robin across the 8 cores (data parallel), pad
    each segment to a fixed per-core cap so all cores run one SPMD program.
  - Device: activations kept transposed [feature, row] so every matmul
    uses the weights in native [in, out] layout as the stationary operand
    (out = W_tile.T @ xT_tile). bf16 matmuls, f32 PSUM accumulation,
    tanh/sigmoid on ScalarE with fused bias, gating product on VectorE.
    Input x-stream DMAs ride the Sync HWDGE ring, weights ride the
    Scalar HWDGE ring, outputs ride GpSimd SWDGE — three independent
    issue paths so the weight preload doesn't stall the first blocks.
  - Host: scatter device outputs back to full [B, H] f32 (g2 rows stay 0).
"""

import sys

try:
    import concourse  # noqa: F401
except ImportError:
    sys.path.insert(0, "/opt/trn_rl_repo")

import numpy as np
import ml_dtypes

import concourse.tile as tile
from concourse import bacc, mybir
from concourse.bass_utils import run_bass_kernel_spmd

B = 32768
H = 512
NCORES = 8
BC = B // NCORES  # per-core shard of the mask output
DEFAULT_CAP = 1408  # per-core per-branch row capacity (B/3/8 = 1365.3 avg)

BF16 = mybir.dt.bfloat16
F32 = mybir.dt.float32
I32 = mybir.dt.int32

# biases stacked [8, 512] in this order
BIAS_ORDER = ["bl1h", "bl1g", "bl2h", "bl2g", "br1h", "br1g", "br2h", "br2g"]

_PROGRAM_CACHE = {}
LAST_RESULT = None  # BassKernelResults of the most recent kernel() call


def _blocks(cap):
    out = []
    rem = cap
    while rem > 0:
        b = 512 if rem >= 576 else rem
        out.append(b)
        rem -= b
    return out


def build_program(cap0, cap1):
    """Emit + compile the SPMD program for per-branch caps (cap0, cap1)."""
    nc = bacc.Bacc("TRN2", target_bir_lowering=False, debug=False,
                   num_devices=NCORES)

    ncols = cap0 + cap1
    x1t_d = nc.dram_tensor("x1t", [3 * H, ncols], BF16, kind="ExternalInput")
    lt_d = nc.dram_tensor("lt", [H, cap1], BF16, kind="ExternalInput")
    grp_d = nc.dram_tensor("grp", [BC], I32, kind="ExternalInput")
    bias_d = nc.dram_tensor("bias8", [8, H], F32, kind="ExternalInput")

    w_shapes = [("wl1h", 3 * H), ("wl1g", 3 * H),
                ("wl2h", H), ("wl2g", H),
                ("wr1h", 3 * H), ("wr1g", 3 * H),
                ("wr2h", 2 * H), ("wr2g", 2 * H)]
    w_d = {}
    for name, kdim in w_shapes:
        w_d[name] = nc.dram_tensor(name, [kdim, H], BF16, kind="ExternalInput")

    outt_d = nc.dram_tensor("outt", [H, ncols], F32, kind="ExternalOutput")
    fin_d = nc.dram_tensor("fin", [BC], I32, kind="ExternalOutput")

    with tile.TileContext(nc) as tc:
        with (
            tc.tile_pool(name="wsb", bufs=1) as wpool,
            tc.tile_pool(name="bsb", bufs=1) as bpool,
            tc.tile_pool(name="xsb", bufs=3) as xpool,
            tc.tile_pool(name="hsb", bufs=2) as hpool,
            tc.tile_pool(name="act", bufs=3) as apool,
            tc.tile_pool(name="acth", bufs=6) as ahpool,
            tc.tile_pool(name="osb", bufs=6) as opool,
            tc.tile_pool(name="msc", bufs=1) as mpool,
            tc.tile_pool(name="ps", bufs=8, space="PSUM") as pspool,
        ):
            # --- persistent weights / biases -------------------------------
            # Weights ride the Scalar HWDGE ring, the x-stream rides the
            # Sync ring, outputs ride GpSimd SWDGE. The first matmuls only
            # gate on bias8 + first half of wl1h + first half of x-block-0;
            # explicit deps below keep later DMAs from stealing HBM
            # bandwidth during that window.
            b_sb = bpool.tile([128, 8, 4], F32, tag="bias8")
            nc.scalar.dma_start(
                out=b_sb[:], in_=bias_d.rearrange("b (j p) -> p b j", p=128)
            )
            bias_ap = {n: b_sb[:, i, :] for i, n in enumerate(BIAS_ORDER)}

            # wl1h is split in two kt-halves so the very first matmuls can
            # start after ~1.5 MB instead of ~3 MB of DMA.
            w_sb = {}      # name -> list of (tile, kt_start, nkt)
            w_dma = {}     # name -> last dma instruction
            for name, kdim in w_shapes:
                nk = kdim // 128
                if name == "wl1h":
                    ta = wpool.tile([128, nk // 2, H], BF16, tag="w_wl1h_a")
                    tb = wpool.tile([128, nk - nk // 2, H], BF16,
                                    tag="w_wl1h_b")
                    w_sb[name] = [(ta, 0, nk // 2), (tb, nk // 2, nk)]
                else:
                    t = wpool.tile([128, nk, H], BF16, tag=f"w_{name}")
                    w_sb[name] = [(t, 0, nk)]

            def _load_w(name):
                src = w_d[name].rearrange("(kt p) f -> p kt f", p=128)
                for t, k0, k1 in w_sb[name]:
                    w_dma[name] = nc.scalar.dma_start(
                        out=t[:], in_=src[:, k0:k1, :])
                return w_dma[name]

            def _wsl(name, kt, ft):
                for t, k0, k1 in w_sb[name]:
                    if k0 <= kt < k1:
                        return t[:, kt - k0, ft * 128:(ft + 1) * 128]
                raise AssertionError

            _d_wl1h = _load_w("wl1h")

            # --- finished mask (GpSimd SWDGE) ------------------------------
            gt = mpool.tile([128, BC // 128], I32, tag="grp")
            nc.gpsimd.dma_start(
                out=gt[:], in_=grp_d.rearrange("(p j) -> p j", p=128)
            )
            ft_ = mpool.tile([128, BC // 128], I32, tag="fin")
            nc.vector.tensor_scalar(ft_[:], gt[:], 2, None,
                                    op0=mybir.AluOpType.is_equal)
            nc.gpsimd.dma_start(
                out=fin_d.rearrange("(p j) -> p j", p=128), in_=ft_[:]
            )

            x1t_r = x1t_d.rearrange("(kt p) n -> p kt n", p=128)
            lt_r = lt_d.rearrange("(kt p) n -> p kt n", p=128)
            outt_r = outt_d.rearrange("(ft p) n -> p ft n", p=128)

            def branch(col0, cap, w1h, w1g, b1h, b1g, w2h, w2g, b2h, b2g,
                       with_lt, deferred_w=(), split_first=False):
                deferred_w = list(deferred_w)
                c0 = 0
                first = True
                for rblk in _blocks(cap):
                    cs = slice(col0 + c0, col0 + c0 + rblk)
                    if split_first and first:
                        xa = xpool.tile([128, 6, rblk], BF16, tag="xa")
                        nc.sync.dma_start(out=xa[:], in_=x1t_r[:, 0:6, cs])
                        xb = xpool.tile([128, 6, rblk], BF16, tag="xb")
                        dxl = nc.sync.dma_start(out=xb[:],
                                                in_=x1t_r[:, 6:12, cs])
                        xparts = [(xa, 0, 6), (xb, 6, 12)]
                        _st["x0_dma"] = dxl
                    else:
                        x_sb = xpool.tile([128, 12, rblk], BF16, tag="x")
                        dxl = nc.sync.dma_start(out=x_sb[:],
                                                in_=x1t_r[:, :, cs])
                        if _st.get("hold_x"):
                            tile.add_dep_helper(
                                dxl.ins, _st["hold_x"].ins, sync=True,
                                reason="defer x prefetch behind wl1h load")
                        xparts = [(x_sb, 0, 12)]
                    first = False

                    def _xsl(kt):
                        for t, k0, k1 in xparts:
                            if k0 <= kt < k1:
                                return t[:, kt - k0, :]
                        raise AssertionError

                    while deferred_w:
                        wname = deferred_w.pop(0)
                        dw = _load_w(wname)
                        if wname == "wl1g":
                            _st["hold_x"] = dw

                    h_sb = hpool.tile([128, 4, rblk], BF16, tag="h")
                    phs, ths = [], []
                    for ft in range(4):
                        ph = pspool.tile([128, rblk], F32, tag="ps")
                        for kt in range(12):
                            nc.tensor.matmul(
                                ph[:], _wsl(w1h, kt, ft), _xsl(kt),
                                start=(kt == 0), stop=(kt == 11),
                            )
                        th = ahpool.tile([128, rblk], BF16, tag="th")
                        nc.scalar.activation(
                            th[:], ph[:], mybir.ActivationFunctionType.Tanh,
                            bias=b1h[:, ft:ft + 1])
                        ths.append(th)
                    for ft in range(4):
                        pg = pspool.tile([128, rblk], F32, tag="ps")
                        for kt in range(12):
                            nc.tensor.matmul(
                                pg[:], _wsl(w1g, kt, ft), _xsl(kt),
                                start=(kt == 0), stop=(kt == 11),
                            )
                        sg = apool.tile([128, rblk], BF16, tag="sg")
                        nc.scalar.activation(
                            sg[:], pg[:], mybir.ActivationFunctionType.Sigmoid,
                            bias=b1g[:, ft:ft + 1])
                        nc.vector.tensor_mul(h_sb[:, ft, :], ths[ft][:], sg[:])

                    if with_lt:
                        lt_sb = xpool.tile([128, 4, rblk], BF16, tag="ltx")
                        dlt = nc.sync.dma_start(
                            out=lt_sb[:],
                            in_=lt_r[:, :, c0: c0 + rblk],
                        )
                        if _st.get("hold_x"):
                            tile.add_dep_helper(
                                dlt.ins, _st["hold_x"].ins, sync=True,
                                reason="defer lt prefetch behind wl1h load")
                    nk2 = 8 if with_lt else 4
                    for ft in range(4):
                        ph = pspool.tile([128, rblk], F32, tag="ps")
                        for kt in range(nk2):
                            rhs = (h_sb[:, kt, :] if kt < 4
                                   else lt_sb[:, kt - 4, :])
                            nc.tensor.matmul(
                                ph[:], _wsl(w2h, kt, ft), rhs,
                                start=(kt == 0), stop=(kt == nk2 - 1),
                            )
                        pg = pspool.tile([128, rblk], F32, tag="ps")
                        for kt in range(nk2):
                            rhs = (h_sb[:, kt, :] if kt < 4
                                   else lt_sb[:, kt - 4, :])
                            nc.tensor.matmul(
                                pg[:], _wsl(w2g, kt, ft), rhs,
                                start=(kt == 0), stop=(kt == nk2 - 1),
                            )
                        th = apool.tile([128, rblk], F32, tag="th2")
                        sg = apool.tile([128, rblk], F32, tag="sg2")
                        nc.scalar.activation(
                            th[:], ph[:], mybir.ActivationFunctionType.Tanh,
                            bias=b2h[:, ft:ft + 1])
                        nc.scalar.activation(
                            sg[:], pg[:], mybir.ActivationFunctionType.Sigmoid,
                            bias=b2g[:, ft:ft + 1])
                        o_sb = opool.tile([128, rblk], F32, tag="o")
                        nc.vector.tensor_mul(o_sb[:], th[:], sg[:])
                        nc.gpsimd.dma_start(
                            out=outt_r[:, ft, cs],
                            in_=o_sb[:],
                        )
                    c0 += rblk

            _st = {"hold_x": _d_wl1h}
            branch(0, cap0,
                   "wl1h", "wl1g", bias_ap["bl1h"], bias_ap["bl1g"],
                   "wl2h", "wl2g", bias_ap["bl2h"], bias_ap["bl2g"],
                   with_lt=False, deferred_w=["wl1g", "wl2h", "wl2g"],
                   split_first=True)
            branch(cap0, cap1,
                   "wr1h", "wr1g", bias_ap["br1h"], bias_ap["br1g"],
                   "wr2h", "wr2g", bias_ap["br2h"], bias_ap["br2g"],
                   with_lt=True,
                   deferred_w=["wr1h", "wr1g", "wr2h", "wr2g"])

    nc.compile()
    return nc


def _get_program(cap0, cap1):
    key = (cap0, cap1)
    if key not in _PROGRAM_CACHE:
        _PROGRAM_CACHE[key] = build_program(cap0, cap1)
    return _PROGRAM_CACHE[key]


def _roundup(x, m):
    return ((x + m - 1) // m) * m


def kernel(node_hidden, node_context, label_embedding, left_embedding, group,
           Wl1h, bl1h, Wl1g, bl1g, Wl2h, bl2h, Wl2g, bl2g,
           Wr1h, br1h, Wr1g, br1g, Wr2h, br2h, Wr2g, br2g,
           trace=False, trace_kwargs=None):
    global LAST_RESULT
    group = np.asarray(group)
    idx0 = np.flatnonzero(group == 0)
    idx1 = np.flatnonzero(group == 1)
    per0 = [idx0[c::NCORES] for c in range(NCORES)]
    per1 = [idx1[c::NCORES] for c in range(NCORES)]
    need0 = max(len(p) for p in per0)
    need1 = max(len(p) for p in per1)
    cap0 = DEFAULT_CAP if need0 <= DEFAULT_CAP else _roundup(need0, 64)
    cap1 = DEFAULT_CAP if need1 <= DEFAULT_CAP else _roundup(need1, 64)

    nc = _get_program(cap0, cap1)

    bf = ml_dtypes.bfloat16
    xcat = np.concatenate(
        [np.asarray(node_hidden), np.asarray(node_context),
         np.asarray(label_embedding)], axis=1)  # [B, 3H] f32
    lemb = np.asarray(left_embedding)

    shared = {
        "wl1h": np.ascontiguousarray(Wl1h).astype(bf),
        "wl1g": np.ascontiguousarray(Wl1g).astype(bf),
        "wr1h": np.ascontiguousarray(Wr1h).astype(bf),
        "wr1g": np.ascontiguousarray(Wr1g).astype(bf),
        "wl2h": np.ascontiguousarray(Wl2h).astype(bf),
        "wl2g": np.ascontiguousarray(Wl2g).astype(bf),
        "wr2h": np.ascontiguousarray(Wr2h).astype(bf),
        "wr2g": np.ascontiguousarray(Wr2g).astype(bf),
        "bias8": np.ascontiguousarray(np.stack(
            [bl1h, bl1g, bl2h, bl2g, br1h, br1g, br2h, br2g]),
            dtype=np.float32),
    }

    in_maps = []
    for c in range(NCORES):
        rows0 = np.zeros(cap0, dtype=np.int64)
        rows0[:len(per0[c])] = per0[c]
        rows1 = np.zeros(cap1, dtype=np.int64)
        rows1[:len(per1[c])] = per1[c]
        rows = np.concatenate([rows0, rows1])
        x1t = np.ascontiguousarray(xcat[rows].T).astype(bf)  # [3H, ncols]
        lt = np.ascontiguousarray(lemb[rows1].T).astype(bf)  # [H, cap1]
        m = dict(shared)
        m["x1t"] = x1t
        m["lt"] = lt
        m["grp"] = np.ascontiguousarray(group[c * BC:(c + 1) * BC],
                                        dtype=np.int32)
        in_maps.append(m)

    res = run_bass_kernel_spmd(nc, in_maps, list(range(NCORES)),
                               trace=trace, **(trace_kwargs or {}))
    LAST_RESULT = res

    children = np.zeros((B, H), dtype=np.float32)
    finished = np.empty(B, dtype=np.int32)
    for c in range(NCORES):
        outt = res.results[c]["outt"]  # [H, ncols] f32
        outr = outt.T  # [ncols, H]
        if len(per0[c]):
            children[per0[c]] = outr[:len(per0[c])]
        if len(per1[c]):
            children[per1[c]] = outr[cap0:cap0 + len(per1[c])]
        finished[c * BC:(c + 1) * BC] = res.results[c]["fin"]
    return children, finished


# revision 17
# speedup vs baseline: 1.4943x; 1.4368x over previous
"""Trainium2 Bass kernel for nn_DecomposeModel (gated 2-layer MLP decompose).

Strategy:
  - Host: sort rows by group. Only group==0 rows need the left GateNN,
    only group==1 rows need the right GateNN, group==2 rows output zero.
    Deal g0/g1 rows round-robin across the 8 cores (data parallel), pad
    each segment to a fixed per-core cap so all cores run one SPMD program.
  - Device: activations kept transposed [feature, row] so every matmul
    uses the weights in native [in, out] layout as the stationary operand
    (out = W_tile.T @ xT_tile). bf16 matmuls, f32 PSUM accumulation,
    tanh/sigmoid on ScalarE with fused bias, gating product on VectorE.
    Input x-stream DMAs ride the Sync HWDGE ring, weights ride the
    Scalar HWDGE ring, outputs ride GpSimd SWDGE — three independent
    issue paths so the weight preload doesn't stall the first blocks.
  - Host: scatter device outputs back to full [B, H] f32 (g2 rows stay 0).
"""

import os
import sys

try:
    import concourse  # noqa: F401
except ImportError:
    sys.path.insert(0, "/opt/trn_rl_repo")

import numpy as np
import ml_dtypes

import concourse.tile as tile
from concourse import bacc, mybir
from concourse.bass_utils import run_bass_kernel_spmd

B = 32768
H = 512
NCORES = 8
BC = B // NCORES  # per-core shard of the mask output
DEFAULT_CAP = 1408  # per-core per-branch row capacity (B/3/8 = 1365.3 avg)
# fp8(e4m3) layer-1 matmuls with DoubleRow: ~1.5x faster tensor-engine
# phase, at ~3x the (still small) absolute error of bf16.
FP8_L1 = os.environ.get("KERNEL_FP8", "0") == "1"

BF16 = mybir.dt.bfloat16
F32 = mybir.dt.float32
I32 = mybir.dt.int32

# biases stacked [8, 512] in this order
BIAS_ORDER = ["bl1h", "bl1g", "bl2h", "bl2g", "br1h", "br1g", "br2h", "br2g"]

_PROGRAM_CACHE = {}
LAST_RESULT = None  # BassKernelResults of the most recent kernel() call


def _blocks(cap):
    out = []
    rem = cap
    while rem > 0:
        b = 512 if rem >= 576 else rem
        out.append(b)
        rem -= b
    return out


def build_program(cap0, cap1, fp8_l1=False):
    """Emit + compile the SPMD program for per-branch caps (cap0, cap1)."""
    nc = bacc.Bacc("TRN2", target_bir_lowering=False, debug=False,
                   num_devices=NCORES)

    L1DT = mybir.dt.float8e4 if fp8_l1 else BF16
    # fp8: layer-1 weights are pre-scaled by 64 on the host so they sit in
    # e4m3's normal range (raw values ~±0.026 are subnormal); descale is
    # free via the activation scale field.
    l1_scale = (1.0 / 64.0) if fp8_l1 else 1.0
    ncols = cap0 + cap1
    x1t_d = nc.dram_tensor("x1t", [3 * H, ncols], L1DT, kind="ExternalInput")
    lt_d = nc.dram_tensor("lt", [H, cap1], BF16, kind="ExternalInput")
    grp_d = nc.dram_tensor("grp", [BC], I32, kind="ExternalInput")
    bias_d = nc.dram_tensor("bias8", [8, H], F32, kind="ExternalInput")

    w_shapes = [("wl1h", 3 * H), ("wl1g", 3 * H),
                ("wl2h", H), ("wl2g", H),
                ("wr1h", 3 * H), ("wr1g", 3 * H),
                ("wr2h", 2 * H), ("wr2g", 2 * H)]
    w_d = {}
    for name, kdim in w_shapes:
        wdt = L1DT if name in ("wl1h", "wl1g", "wr1h", "wr1g") else BF16
        w_d[name] = nc.dram_tensor(name, [kdim, H], wdt, kind="ExternalInput")

    outt_d = nc.dram_tensor("outt", [H, ncols], F32, kind="ExternalOutput")
    fin_d = nc.dram_tensor("fin", [BC], I32, kind="ExternalOutput")

    with tile.TileContext(nc) as tc:
        with (
            tc.tile_pool(name="wsb", bufs=1) as wpool,
            tc.tile_pool(name="bsb", bufs=1) as bpool,
            tc.tile_pool(name="xsb", bufs=3) as xpool,
            tc.tile_pool(name="hsb", bufs=2) as hpool,
            tc.tile_pool(name="act", bufs=3) as apool,
            tc.tile_pool(name="acth", bufs=6) as ahpool,
            tc.tile_pool(name="osb", bufs=6) as opool,
            tc.tile_pool(name="msc", bufs=1) as mpool,
            tc.tile_pool(name="ps", bufs=8, space="PSUM") as pspool,
        ):
            # --- persistent weights / biases -------------------------------
            # Weights ride the Scalar HWDGE ring, the x-stream rides the
            # Sync ring, outputs ride GpSimd SWDGE. The first matmuls only
            # gate on bias8 + first half of wl1h + first half of x-block-0;
            # explicit deps below keep later DMAs from stealing HBM
            # bandwidth during that window.
            b_sb = bpool.tile([128, 8, 4], F32, tag="bias8")
            nc.scalar.dma_start(
                out=b_sb[:], in_=bias_d.rearrange("b (j p) -> p b j", p=128)
            )
            bias_ap = {n: b_sb[:, i, :] for i, n in enumerate(BIAS_ORDER)}

            # wl1h is split in two kt-halves so the very first matmuls can
            # start after ~1.5 MB instead of ~3 MB of DMA.
            w_sb = {}      # name -> list of (tile, kt_start, nkt)
            w_dma = {}     # name -> last dma instruction
            for name, kdim in w_shapes:
                nk = kdim // 128
                wdt = L1DT if name in ("wl1h", "wl1g", "wr1h", "wr1g") else BF16
                if name == "wl1h":
                    ta = wpool.tile([128, nk // 2, H], wdt, tag="w_wl1h_a")
                    tb = wpool.tile([128, nk - nk // 2, H], wdt,
                                    tag="w_wl1h_b")
                    w_sb[name] = [(ta, 0, nk // 2), (tb, nk // 2, nk)]
                else:
                    t = wpool.tile([128, nk, H], wdt, tag=f"w_{name}")
                    w_sb[name] = [(t, 0, nk)]

            def _load_w(name):
                src = w_d[name].rearrange("(kt p) f -> p kt f", p=128)
                for t, k0, k1 in w_sb[name]:
                    w_dma[name] = nc.scalar.dma_start(
                        out=t[:], in_=src[:, k0:k1, :])
                return w_dma[name]

            def _wsl(name, kt, ft, span=1):
                for t, k0, k1 in w_sb[name]:
                    if k0 <= kt and kt + span <= k1:
                        if span == 1:
                            return t[:, kt - k0, ft * 128:(ft + 1) * 128]
                        return t[:, kt - k0:kt - k0 + span,
                                 ft * 128:(ft + 1) * 128]
                raise AssertionError

            _d_wl1h = _load_w("wl1h")

            # --- finished mask (GpSimd SWDGE) ------------------------------
            gt = mpool.tile([128, BC // 128], I32, tag="grp")
            nc.gpsimd.dma_start(
                out=gt[:], in_=grp_d.rearrange("(p j) -> p j", p=128)
            )
            ft_ = mpool.tile([128, BC // 128], I32, tag="fin")
            nc.vector.tensor_scalar(ft_[:], gt[:], 2, None,
                                    op0=mybir.AluOpType.is_equal)
            nc.gpsimd.dma_start(
                out=fin_d.rearrange("(p j) -> p j", p=128), in_=ft_[:]
            )

            x1t_r = x1t_d.rearrange("(kt p) n -> p kt n", p=128)
            lt_r = lt_d.rearrange("(kt p) n -> p kt n", p=128)
            outt_r = outt_d.rearrange("(ft p) n -> p ft n", p=128)

            def branch(col0, cap, w1h, w1g, b1h, b1g, w2h, w2g, b2h, b2g,
                       with_lt, deferred_w=(), split_first=False):
                deferred_w = list(deferred_w)
                c0 = 0
                first = True
                for rblk in _blocks(cap):
                    cs = slice(col0 + c0, col0 + c0 + rblk)
                    if split_first and first:
                        xa = xpool.tile([128, 6, rblk], L1DT, tag="xa")
                        nc.sync.dma_start(out=xa[:], in_=x1t_r[:, 0:6, cs])
                        xb = xpool.tile([128, 6, rblk], L1DT, tag="xb")
                        dxl = nc.sync.dma_start(out=xb[:],
                                                in_=x1t_r[:, 6:12, cs])
                        xparts = [(xa, 0, 6), (xb, 6, 12)]
                        _st["x0_dma"] = dxl
                    else:
                        x_sb = xpool.tile([128, 12, rblk], L1DT, tag="x")
                        dxl = nc.sync.dma_start(out=x_sb[:],
                                                in_=x1t_r[:, :, cs])
                        if _st.get("hold_x"):
                            tile.add_dep_helper(
                                dxl.ins, _st["hold_x"].ins, sync=True,
                                reason="defer x prefetch behind wl1h load")
                        xparts = [(x_sb, 0, 12)]
                    first = False

                    def _xsl(kt, span=1):
                        for t, k0, k1 in xparts:
                            if k0 <= kt and kt + span <= k1:
                                if span == 1:
                                    return t[:, kt - k0, :]
                                return t[:, kt - k0:kt - k0 + span, :]
                        raise AssertionError

                    while deferred_w:
                        wname = deferred_w.pop(0)
                        dw = _load_w(wname)
                        if wname == "wl1g":
                            _st["hold_x"] = dw

                    h_sb = hpool.tile([128, 4, rblk], BF16, tag="h")
                    kstep = 2 if fp8_l1 else 1
                    pmode = (mybir.MatmulPerfMode.DoubleRow if fp8_l1
                             else None)
                    phs, ths = [], []
                    for ft in range(4):
                        ph = pspool.tile([128, rblk], F32, tag="ps")
                        for kt in range(0, 12, kstep):
                            nc.tensor.matmul(
                                ph[:], _wsl(w1h, kt, ft, kstep),
                                _xsl(kt, kstep),
                                start=(kt == 0), stop=(kt == 12 - kstep),
                                perf_mode=pmode,
                            )
                        th = ahpool.tile([128, rblk], BF16, tag="th")
                        nc.scalar.activation(
                            th[:], ph[:], mybir.ActivationFunctionType.Tanh,
                            bias=b1h[:, ft:ft + 1], scale=l1_scale)
                        ths.append(th)
                    for ft in range(4):
                        pg = pspool.tile([128, rblk], F32, tag="ps")
                        for kt in range(0, 12, kstep):
                            nc.tensor.matmul(
                                pg[:], _wsl(w1g, kt, ft, kstep),
                                _xsl(kt, kstep),
                                start=(kt == 0), stop=(kt == 12 - kstep),
                                perf_mode=pmode,
                            )
                        sg = apool.tile([128, rblk], BF16, tag="sg")
                        nc.scalar.activation(
                            sg[:], pg[:], mybir.ActivationFunctionType.Sigmoid,
                            bias=b1g[:, ft:ft + 1], scale=l1_scale)
                        nc.vector.tensor_mul(h_sb[:, ft, :], ths[ft][:], sg[:])

                    if with_lt:
                        lt_sb = xpool.tile([128, 4, rblk], BF16, tag="ltx")
                        dlt = nc.sync.dma_start(
                            out=lt_sb[:],
                            in_=lt_r[:, :, c0: c0 + rblk],
                        )
                        if _st.get("hold_x"):
                            tile.add_dep_helper(
                                dlt.ins, _st["hold_x"].ins, sync=True,
                                reason="defer lt prefetch behind wl1h load")
                    nk2 = 8 if with_lt else 4
                    for ft in range(4):
                        ph = pspool.tile([128, rblk], F32, tag="ps")
                        for kt in range(nk2):
                            rhs = (h_sb[:, kt, :] if kt < 4
                                   else lt_sb[:, kt - 4, :])
                            nc.tensor.matmul(
                                ph[:], _wsl(w2h, kt, ft), rhs,
                                start=(kt == 0), stop=(kt == nk2 - 1),
                            )
                        pg = pspool.tile([128, rblk], F32, tag="ps")
                        for kt in range(nk2):
                            rhs = (h_sb[:, kt, :] if kt < 4
                                   else lt_sb[:, kt - 4, :])
                            nc.tensor.matmul(
                                pg[:], _wsl(w2g, kt, ft), rhs,
                                start=(kt == 0), stop=(kt == nk2 - 1),
                            )
                        th = apool.tile([128, rblk], F32, tag="th2")
                        sg = apool.tile([128, rblk], F32, tag="sg2")
                        nc.scalar.activation(
                            th[:], ph[:], mybir.ActivationFunctionType.Tanh,
                            bias=b2h[:, ft:ft + 1])
                        nc.scalar.activation(
                            sg[:], pg[:], mybir.ActivationFunctionType.Sigmoid,
                            bias=b2g[:, ft:ft + 1])
                        o_sb = opool.tile([128, rblk], F32, tag="o")
                        nc.vector.tensor_mul(o_sb[:], th[:], sg[:])
                        nc.gpsimd.dma_start(
                            out=outt_r[:, ft, cs],
                            in_=o_sb[:],
                        )
                    c0 += rblk

            _st = {"hold_x": _d_wl1h}
            branch(0, cap0,
                   "wl1h", "wl1g", bias_ap["bl1h"], bias_ap["bl1g"],
                   "wl2h", "wl2g", bias_ap["bl2h"], bias_ap["bl2g"],
                   with_lt=False, deferred_w=["wl1g", "wl2h", "wl2g"],
                   split_first=True)
            branch(cap0, cap1,
                   "wr1h", "wr1g", bias_ap["br1h"], bias_ap["br1g"],
                   "wr2h", "wr2g", bias_ap["br2h"], bias_ap["br2g"],
                   with_lt=True,
                   deferred_w=["wr1h", "wr1g", "wr2h", "wr2g"])

    nc.compile()
    return nc


def _get_program(cap0, cap1, fp8_l1):
    key = (cap0, cap1, fp8_l1)
    if key not in _PROGRAM_CACHE:
        _PROGRAM_CACHE[key] = build_program(cap0, cap1, fp8_l1)
    return _PROGRAM_CACHE[key]


def _roundup(x, m):
    return ((x + m - 1) // m) * m


def kernel(node_hidden, node_context, label_embedding, left_embedding, group,
           Wl1h, bl1h, Wl1g, bl1g, Wl2h, bl2h, Wl2g, bl2g,
           Wr1h, br1h, Wr1g, br1g, Wr2h, br2h, Wr2g, br2g,
           trace=False, trace_kwargs=None):
    global LAST_RESULT
    group = np.asarray(group)
    idx0 = np.flatnonzero(group == 0)
    idx1 = np.flatnonzero(group == 1)
    per0 = [idx0[c::NCORES] for c in range(NCORES)]
    per1 = [idx1[c::NCORES] for c in range(NCORES)]
    need0 = max(len(p) for p in per0)
    need1 = max(len(p) for p in per1)
    cap0 = DEFAULT_CAP if need0 <= DEFAULT_CAP else _roundup(need0, 64)
    cap1 = DEFAULT_CAP if need1 <= DEFAULT_CAP else _roundup(need1, 64)

    fp8_l1 = FP8_L1
    nc = _get_program(cap0, cap1, fp8_l1)

    bf = ml_dtypes.bfloat16
    l1dt = ml_dtypes.float8_e4m3fn if fp8_l1 else bf
    xcat = np.concatenate(
        [np.asarray(node_hidden), np.asarray(node_context),
         np.asarray(label_embedding)], axis=1)  # [B, 3H] f32
    lemb = np.asarray(left_embedding)

    wsc = 64.0 if fp8_l1 else 1.0
    shared = {
        "wl1h": np.ascontiguousarray(np.asarray(Wl1h) * wsc).astype(l1dt),
        "wl1g": np.ascontiguousarray(np.asarray(Wl1g) * wsc).astype(l1dt),
        "wr1h": np.ascontiguousarray(np.asarray(Wr1h) * wsc).astype(l1dt),
        "wr1g": np.ascontiguousarray(np.asarray(Wr1g) * wsc).astype(l1dt),
        "wl2h": np.ascontiguousarray(Wl2h).astype(bf),
        "wl2g": np.ascontiguousarray(Wl2g).astype(bf),
        "wr2h": np.ascontiguousarray(Wr2h).astype(bf),
        "wr2g": np.ascontiguousarray(Wr2g).astype(bf),
        "bias8": np.ascontiguousarray(np.stack(
            [bl1h, bl1g, bl2h, bl2g, br1h, br1g, br2h, br2g]),
            dtype=np.float32),
    }

    in_maps = []
    for c in range(NCORES):
        rows0 = np.zeros(cap0, dtype=np.int64)
        rows0[:len(per0[c])] = per0[c]
        rows1 = np.zeros(cap1, dtype=np.int64)
        rows1[:len(per1[c])] = per1[c]
        rows = np.concatenate([rows0, rows1])
        x1t = np.ascontiguousarray(xcat[rows].T).astype(l1dt)  # [3H, ncols]
        lt = np.ascontiguousarray(lemb[rows1].T).astype(bf)  # [H, cap1]
        m = dict(shared)
        m["x1t"] = x1t
        m["lt"] = lt
        m["grp"] = np.ascontiguousarray(group[c * BC:(c + 1) * BC],
                                        dtype=np.int32)
        in_maps.append(m)

    res = run_bass_kernel_spmd(nc, in_maps, list(range(NCORES)),
                               trace=trace, **(trace_kwargs or {}))
    LAST_RESULT = res

    children = np.zeros((B, H), dtype=np.float32)
    finished = np.empty(B, dtype=np.int32)
    for c in range(NCORES):
        outt = res.results[c]["outt"]  # [H, ncols] f32
        outr = outt.T  # [ncols, H]
        if len(per0[c]):
            children[per0[c]] = outr[:len(per0[c])]
        if len(per1[c]):
            children[per1[c]] = outr[cap0:cap0 + len(per1[c])]
        finished[c * BC:(c + 1) * BC] = res.results[c]["fin"]
    return children, finished
